# revision 34
# baseline (speedup 1.0000x reference)
"""Trainium2 Bass kernel for nn_Classifier (per-class binary log_softmax head).

Reference computation:
    logits[b, c, t] = x[b, :] @ W[c, t, :] + bias[c, t]      # [B, C, 2]
    out = log_softmax(logits, axis=-1)

Key algebraic reduction: log_softmax over the 2 logits per class depends only
on the difference d = l1 - l0:
    out0 = -softplus(d)
    out1 = d - softplus(d)
where d[b, c] = x[b, :] @ (W[c,1,:] - W[c,0,:]) + (bias[c,1] - bias[c,0]).
This halves the matmul FLOPs vs computing both logits.

Strategy (8 NeuronCores, data-parallel over batch):
  - core i gets x rows [i*2048, (i+1)*2048); W and b are replicated.
  - on-device: dW = W1 - W0 (DVE), PE-transpose to [D, C] bf16;
    db = b1 - b0 folded into PSUM accumulation via a K=1 ones-matmul.
  - per 128-row batch tile: PE-transpose x chunks to [D, 128] bf16 (lhsT),
    matmul accumulate d in PSUM fp32, then ACT softplus + DVE/ACT epilogue
    writes the interleaved [128, 2000] fp32 output tile, DMA'd out (1 MiB).
"""

import os
import sys

for _p in ("/opt/trn_rl_repo", "/root/.axon_site/_ro/trn_rl_repo"):
    if os.path.isdir(_p) and _p not in sys.path:
        sys.path.insert(0, _p)

import numpy as np

import concourse.bass as bass
import concourse.mybir as mybir
import concourse.tile as tile
from concourse import bacc
from concourse.bass_utils import run_bass_kernel_spmd
from concourse.masks import make_identity

def _patch_act_tables():
    """Force Exp and Ln activations into ONE ACT table set.

    The stock table-set assignment puts Exp and Ln in different sets, so
    alternating Exp/Ln reloads the 1.3us ACT function table before every
    activation (~82us serialized on the scalar engine for this kernel).
    natural_log_exp_and_others contains both (at the higher-accuracy
    400-point tables). Removing exp/ln from every OTHER set makes bacc's
    insert_act_table_loads fixpoint assign both to that one set; set ids
    stay aligned with the stock act_info.json, so walrus adopts the
    pre-placed loads unchanged.
    """
    import functools

    import concourse.bacc as _bacc
    import concourse.hw_specs as _hw

    orig = _hw.get_activation_tables

    @functools.cache
    def patched(module_arch):
        exp = mybir.ActivationFunctionType.Exp
        ln = mybir.ActivationFunctionType.Ln
        out = {}
        for name, funcs in orig(module_arch).items():
            if name != "natural_log_exp_and_others":
                funcs = funcs - {exp, ln}
            out[name] = funcs
        return out

    _hw.get_activation_tables = patched
    _bacc.get_activation_tables = patched


_patch_act_tables()


# bump when the compile environment changes semantics: the neuron compile
# cache keys on the BIR bytes, and this tag is embedded in a tensor name so
# the key changes with it.
KERNEL_TAG = "v10"

P = 128
D = 512  # input dim
C = 1000  # num classes
B = 16384  # batch
NCORES = 8
BS = B // NCORES  # 2048 rows per core
BT = BS // P  # 16 batch tiles per core
KC = D // P  # 4 contraction chunks
NCH = 500  # classes per matmul n-chunk (2 chunks; 500 fp32 <= 1 PSUM bank)

F32 = mybir.dt.float32
BF16 = mybir.dt.bfloat16

# matmul operand dtype: bf16 is full PE rate (fp32 costs 2 passes). PSUM
# accumulation is fp32 either way. Set to F32 if accuracy requires it.
MM_DT = BF16


def build_nc():
    nc = bacc.Bacc(None, target_bir_lowering=False)
    x_in = nc.dram_tensor("x", [BS, D], F32, kind="ExternalInput").ap()
    w_in = nc.dram_tensor("w", [2 * C, D], F32, kind="ExternalInput").ap()
    b_in = nc.dram_tensor("b", [1, 2 * C], F32, kind="ExternalInput").ap()
    out = nc.dram_tensor("out", [BS, 2 * C], F32, kind="ExternalOutput").ap()

    with tile.TileContext(nc) as tc:
        with (
            tc.tile_pool(name="const", bufs=1) as const,
            tc.tile_pool(name="wstage", bufs=4) as wstage,
            tc.tile_pool(name="dwstage", bufs=2) as dwstage,
            tc.tile_pool(name="xstage", bufs=4) as xstage,
            tc.tile_pool(name="xtp", bufs=3) as xtp,
            tc.tile_pool(name="spool", bufs=4) as spool,
            tc.tile_pool(name="opool", bufs=3) as opool,
            tc.tile_pool(name="tpsum", bufs=2, space="PSUM") as tpsum,
            tc.tile_pool(name="mpsum", bufs=3, space="PSUM") as mpsum,
        ):
            identity = const.tile([P, P], MM_DT, name=f"identity_{KERNEL_TAG}")
            make_identity(nc, identity)

            ones_row = const.tile([1, P], MM_DT)
            nc.vector.memset(ones_row, 1.0)

            # ---- bias prep: db[c] = b[c,1] - b[c,0] ----
            btile = const.tile([1, 2 * C], F32)
            nc.sync.dma_start(out=btile, in_=b_in)
            b3 = btile.rearrange("p (c t) -> p t c", t=2)  # [1, 2, C] view
            db_f = const.tile([1, C], F32)
            nc.gpsimd.tensor_sub(db_f, b3[:, 1, :], b3[:, 0, :])
            db = const.tile([1, C], MM_DT)
            nc.gpsimd.tensor_copy(out=db, in_=db_f)

            # ---- dW prep: dwt[j][d_chunk][:, c] = (W1 - W0).T in MM_DT ----
            # split by n-chunk so chunk-0 matmuls start after half the W load
            dwt0 = const.tile([P, KC, NCH], MM_DT)
            dwt1 = const.tile([P, KC, NCH], MM_DT)
            dwts = [dwt0, dwt1]
            w3 = w_in.rearrange("(c t) d -> t c d", t=2)  # [2, C, D] view
            for wt in range((C + P - 1) // P):  # 8 row tiles (last = 104 rows)
                r0 = wt * P
                rows = min(P, C - r0)
                w1t = wstage.tile([P, D], F32, tag="wst")
                w0t = wstage.tile([P, D], F32, tag="wst")
                nc.sync.dma_start(out=w1t[:rows], in_=w3[1, r0 : r0 + rows, :])
                nc.sync.dma_start(out=w0t[:rows], in_=w3[0, r0 : r0 + rows, :])
                # fused sub + bf16 cast (output dtype converts on write)
                dwb = dwstage.tile([P, D], MM_DT, tag="dwb")
                nc.gpsimd.tensor_sub(dwb[:rows], w1t[:rows], w0t[:rows])
                # 4 transposed chunks share one PSUM bank -> single wide cast
                pt = tpsum.tile([P, KC * P], MM_DT, tag="tp")
                for k in range(KC):
                    nc.tensor.transpose(
                        pt[:, k * P : k * P + rows],
                        dwb[:rows, k * P : (k + 1) * P],
                        identity[:rows, :rows],
                    )
                ptv = pt.rearrange("p (k b) -> p k b", k=KC)
                # scatter the [r0, r0+rows) class range into dwt0/dwt1
                for j in (0, 1):
                    lo = max(r0, j * NCH)
                    hi = min(r0 + rows, (j + 1) * NCH)
                    if lo < hi:
                        nc.vector.tensor_copy(
                            out=dwts[j][:, :, lo - j * NCH : hi - j * NCH],
                            in_=ptv[:, :, lo - r0 : hi - r0],
                        )

            # ---- main loop over batch tiles ----
            for bt in range(BT):
                # SWDGE (gpsimd) DMA casts f32->bf16 in flight
                xb = xstage.tile([P, D], MM_DT, tag="xb")
                nc.gpsimd.dma_start(
                    out=xb, in_=x_in[bt * P : (bt + 1) * P, :]
                )
                # 4 transposed chunks share one PSUM bank -> single wide cast
                ptx = tpsum.tile([P, KC * P], MM_DT, tag="tp")
                for k in range(KC):
                    nc.tensor.transpose(
                        ptx[:, k * P : (k + 1) * P],
                        xb[:, k * P : (k + 1) * P],
                        identity,
                    )
                xT = xtp.tile([P, KC, P], MM_DT, tag="xT")
                nc.vector.tensor_copy(
                    out=xT.rearrange("p k b -> p (k b)"), in_=ptx
                )

                ot = opool.tile([P, 2 * C], F32)
                # [P, j, c, t] view: class index = j*NCH + c, logit t
                ot4 = ot.rearrange("p (j c t) -> p j c t", j=2, t=2)

                # bank-aligned [P, 2, 512] PSUM tile; each matmul group fills
                # its own 512-wide bank (500 used), the epilogue then runs
                # both halves in single wide ops to amortize fixed overheads.
                dps = mpsum.tile([P, 2, 512], F32, tag="mp")
                for j in range(C // NCH):  # 2 n-chunks of 500 classes
                    c0 = j * NCH
                    dpsj = dps[:, j, :NCH]
                    for k in range(KC):
                        nc.tensor.matmul(
                            dpsj,
                            lhsT=xT[:, k, :],
                            rhs=dwts[j][:, k, :],
                            start=(k == 0),
                            stop=False,
                        )
                    # += ones.T @ db  (broadcasts bias diff across partitions)
                    nc.tensor.matmul(
                        dpsj,
                        lhsT=ones_row,
                        rhs=db[:, c0 : c0 + NCH],
                        start=False,
                        stop=True,
                    )
                # softplus(d) = ln(exp(d) + 1); this toolchain has no direct
                # Softplus ACT table, but natural_log_exp_and_others provides
                # Exp and Ln in one set. |d| <~ 6 so exp is safe.
                dv = dps[:, :, :NCH]  # [P, 2, 500]
                e = spool.tile([P, 2, NCH], F32, tag="sp")
                nc.scalar.activation(e, dv, mybir.ActivationFunctionType.Exp)
                s = spool.tile([P, 2, NCH], F32, tag="sp")
                nc.scalar.activation(
                    s, e, mybir.ActivationFunctionType.Ln, bias=1.0
                )
                # out0 = -s   (strided write into interleaved out tile)
                nc.vector.tensor_scalar_mul(ot4[:, :, :, 0], s, -1.0)
                # out1 = d - s  (strided write)
                nc.vector.tensor_sub(ot4[:, :, :, 1], dv, s)

                nc.sync.dma_start(
                    out=out[bt * P : (bt + 1) * P, :], in_=ot
                )

    nc.finalize()
    return nc


_NC_CACHE = None


def _get_nc():
    global _NC_CACHE
    if _NC_CACHE is None:
        _NC_CACHE = build_nc()
    return _NC_CACHE


def kernel(x, W, b):
    x = np.ascontiguousarray(np.asarray(x, dtype=np.float32))
    W = np.ascontiguousarray(np.asarray(W, dtype=np.float32))
    b = np.ascontiguousarray(np.asarray(b, dtype=np.float32))
    assert x.shape == (B, D) and W.shape == (C, 2, D) and b.shape == (C, 2)

    nc = _get_nc()
    w2d = W.reshape(2 * C, D)
    b2d = b.reshape(1, 2 * C)
    in_maps = [
        {"x": x[i * BS : (i + 1) * BS], "w": w2d, "b": b2d} for i in range(NCORES)
    ]
    res = run_bass_kernel_spmd(nc, in_maps, core_ids=list(range(NCORES)))
    full = np.concatenate([res.results[i]["out"] for i in range(NCORES)], axis=0)
    return full.reshape(B, C, 2)


# revision 36
# speedup vs baseline: 1.1203x; 1.1203x over previous
"""Trainium2 Bass kernel for nn_Classifier (per-class binary log_softmax head).

Reference computation:
    logits[b, c, t] = x[b, :] @ W[c, t, :] + bias[c, t]      # [B, C, 2]
    out = log_softmax(logits, axis=-1)

Key algebraic reduction: log_softmax over the 2 logits per class depends only
on the difference d = l1 - l0:
    out0 = -softplus(d)
    out1 = d - softplus(d)
where d[b, c] = x[b, :] @ (W[c,1,:] - W[c,0,:]) + (bias[c,1] - bias[c,0]).
This halves the matmul FLOPs vs computing both logits.

Strategy (8 NeuronCores, data-parallel over batch):
  - core i gets x rows [i*2048, (i+1)*2048); W and b are replicated.
  - on-device: dW = W1 - W0 (DVE), PE-transpose to [D, C] bf16;
    db = b1 - b0 folded into PSUM accumulation via a K=1 ones-matmul.
  - per 128-row batch tile: PE-transpose x chunks to [D, 128] bf16 (lhsT),
    matmul accumulate d in PSUM fp32, then ACT softplus + DVE/ACT epilogue
    writes the interleaved [128, 2000] fp32 output tile, DMA'd out (1 MiB).
"""

import os
import sys

for _p in ("/opt/trn_rl_repo", "/root/.axon_site/_ro/trn_rl_repo"):
    if os.path.isdir(_p) and _p not in sys.path:
        sys.path.insert(0, _p)

import numpy as np

import concourse.bass as bass
import concourse.mybir as mybir
import concourse.tile as tile
from concourse import bacc
from concourse.bass_utils import run_bass_kernel_spmd
from concourse.masks import make_identity

def _patch_act_tables():
    """Force Exp and Ln activations into ONE ACT table set.

    The stock table-set assignment puts Exp and Ln in different sets, so
    alternating Exp/Ln reloads the 1.3us ACT function table before every
    activation (~82us serialized on the scalar engine for this kernel).
    natural_log_exp_and_others contains both (at the higher-accuracy
    400-point tables). Removing exp/ln from every OTHER set makes bacc's
    insert_act_table_loads fixpoint assign both to that one set; set ids
    stay aligned with the stock act_info.json, so walrus adopts the
    pre-placed loads unchanged.
    """
    import functools

    import concourse.bacc as _bacc
    import concourse.hw_specs as _hw

    orig = _hw.get_activation_tables

    @functools.cache
    def patched(module_arch):
        exp = mybir.ActivationFunctionType.Exp
        ln = mybir.ActivationFunctionType.Ln
        out = {}
        for name, funcs in orig(module_arch).items():
            if name != "natural_log_exp_and_others":
                funcs = funcs - {exp, ln}
            out[name] = funcs
        return out

    _hw.get_activation_tables = patched
    _bacc.get_activation_tables = patched


_patch_act_tables()


# bump when the compile environment changes semantics: the neuron compile
# cache keys on the BIR bytes, and this tag is embedded in a tensor name so
# the key changes with it.
KERNEL_TAG = "v11"

P = 128
D = 512  # input dim
C = 1000  # num classes
B = 16384  # batch
NCORES = 8
BS = B // NCORES  # 2048 rows per core
BT = BS // P  # 16 batch tiles per core
KC = D // P  # 4 contraction chunks
NCH = 500  # classes per matmul n-chunk (2 chunks; 500 fp32 <= 1 PSUM bank)

F32 = mybir.dt.float32
BF16 = mybir.dt.bfloat16

# matmul operand dtype: bf16 is full PE rate (fp32 costs 2 passes). PSUM
# accumulation is fp32 either way. Set to F32 if accuracy requires it.
MM_DT = BF16


def build_nc():
    nc = bacc.Bacc(None, target_bir_lowering=False)
    x_in = nc.dram_tensor("x", [BS, D], F32, kind="ExternalInput").ap()
    w_in = nc.dram_tensor("w", [2 * C, D], F32, kind="ExternalInput").ap()
    b_in = nc.dram_tensor("b", [1, 2 * C], F32, kind="ExternalInput").ap()
    out = nc.dram_tensor("out", [BS, 2 * C], F32, kind="ExternalOutput").ap()

    with tile.TileContext(nc) as tc:
        with (
            tc.tile_pool(name="const", bufs=1) as const,
            tc.tile_pool(name="wstage", bufs=4) as wstage,
            tc.tile_pool(name="dwstage", bufs=2) as dwstage,
            tc.tile_pool(name="xstage", bufs=4) as xstage,
            tc.tile_pool(name="xtp", bufs=3) as xtp,
            tc.tile_pool(name="spool", bufs=4) as spool,
            tc.tile_pool(name="opool", bufs=3) as opool,
            tc.tile_pool(name="tpsum", bufs=2, space="PSUM") as tpsum,
            tc.tile_pool(name="mpsum", bufs=3, space="PSUM") as mpsum,
        ):
            identity = const.tile([P, P], MM_DT, name=f"identity_{KERNEL_TAG}")
            make_identity(nc, identity)

            ones_row = const.tile([1, P], MM_DT)
            nc.vector.memset(ones_row, 1.0)

            # ---- bias prep: db[c] = b[c,1] - b[c,0] ----
            btile = const.tile([1, 2 * C], F32)
            nc.sync.dma_start(out=btile, in_=b_in)
            b3 = btile.rearrange("p (c t) -> p t c", t=2)  # [1, 2, C] view
            db_f = const.tile([1, C], F32)
            nc.gpsimd.tensor_sub(db_f, b3[:, 1, :], b3[:, 0, :])
            db = const.tile([1, C], MM_DT)
            nc.gpsimd.tensor_copy(out=db, in_=db_f)

            # ---- dW prep: dwt[j][d_chunk][:, c] = (W1 - W0).T in MM_DT ----
            # split by n-chunk so chunk-0 matmuls start after half the W load
            dwt0 = const.tile([P, KC, NCH], MM_DT)
            dwt1 = const.tile([P, KC, NCH], MM_DT)
            dwts = [dwt0, dwt1]
            w3 = w_in.rearrange("(c t) d -> t c d", t=2)  # [2, C, D] view
            for wt in range((C + P - 1) // P):  # 8 row tiles (last = 104 rows)
                r0 = wt * P
                rows = min(P, C - r0)
                w1t = wstage.tile([P, D], F32, tag="wst")
                w0t = wstage.tile([P, D], F32, tag="wst")
                nc.sync.dma_start(out=w1t[:rows], in_=w3[1, r0 : r0 + rows, :])
                nc.sync.dma_start(out=w0t[:rows], in_=w3[0, r0 : r0 + rows, :])
                # fused sub + bf16 cast (output dtype converts on write)
                dwb = dwstage.tile([P, D], MM_DT, tag="dwb")
                nc.gpsimd.tensor_sub(dwb[:rows], w1t[:rows], w0t[:rows])
                # 4 transposed chunks share one PSUM bank -> single wide cast
                pt = tpsum.tile([P, KC * P], MM_DT, tag="tp")
                for k in range(KC):
                    nc.tensor.transpose(
                        pt[:, k * P : k * P + rows],
                        dwb[:rows, k * P : (k + 1) * P],
                        identity[:rows, :rows],
                    )
                ptv = pt.rearrange("p (k b) -> p k b", k=KC)
                # scatter the [r0, r0+rows) class range into dwt0/dwt1
                for j in (0, 1):
                    lo = max(r0, j * NCH)
                    hi = min(r0 + rows, (j + 1) * NCH)
                    if lo < hi:
                        nc.vector.tensor_copy(
                            out=dwts[j][:, :, lo - j * NCH : hi - j * NCH],
                            in_=ptv[:, :, lo - r0 : hi - r0],
                        )

            # ---- main loop over batch tiles ----
            for bt in range(BT):
                # scalar-engine HWDGE ring: x loads don't queue behind the
                # 4 MiB W load on the sync ring
                xt_ = xstage.tile([P, D], F32)
                nc.scalar.dma_start(out=xt_, in_=x_in[bt * P : (bt + 1) * P, :])
                xb = xstage.tile([P, D], MM_DT, tag="xb")
                nc.vector.tensor_copy(out=xb, in_=xt_)
                # 4 transposed chunks share one PSUM bank -> single wide cast
                ptx = tpsum.tile([P, KC * P], MM_DT, tag="tp")
                for k in range(KC):
                    nc.tensor.transpose(
                        ptx[:, k * P : (k + 1) * P],
                        xb[:, k * P : (k + 1) * P],
                        identity,
                    )
                xT = xtp.tile([P, KC, P], MM_DT, tag="xT")
                nc.vector.tensor_copy(
                    out=xT.rearrange("p k b -> p (k b)"), in_=ptx
                )

                ot = opool.tile([P, 2 * C], F32)
                # [P, j, c, t] view: class index = j*NCH + c, logit t
                ot4 = ot.rearrange("p (j c t) -> p j c t", j=2, t=2)

                # bank-aligned [P, 2, 512] PSUM tile; each matmul group fills
                # its own 512-wide bank (500 used), the epilogue then runs
                # both halves in single wide ops to amortize fixed overheads.
                dps = mpsum.tile([P, 2, 512], F32, tag="mp")
                for j in range(C // NCH):  # 2 n-chunks of 500 classes
                    c0 = j * NCH
                    dpsj = dps[:, j, :NCH]
                    for k in range(KC):
                        nc.tensor.matmul(
                            dpsj,
                            lhsT=xT[:, k, :],
                            rhs=dwts[j][:, k, :],
                            start=(k == 0),
                            stop=False,
                        )
                    # += ones.T @ db  (broadcasts bias diff across partitions)
                    nc.tensor.matmul(
                        dpsj,
                        lhsT=ones_row,
                        rhs=db[:, c0 : c0 + NCH],
                        start=False,
                        stop=True,
                    )
                # softplus(d) = ln(exp(d) + 1); this toolchain has no direct
                # Softplus ACT table, but natural_log_exp_and_others provides
                # Exp and Ln in one set. |d| <~ 6 so exp is safe.
                dv = dps[:, :, :NCH]  # [P, 2, 500]
                e = spool.tile([P, 2, NCH], F32, tag="sp")
                nc.scalar.activation(e, dv, mybir.ActivationFunctionType.Exp)
                s = spool.tile([P, 2, NCH], F32, tag="sp")
                nc.scalar.activation(
                    s, e, mybir.ActivationFunctionType.Ln, bias=1.0
                )
                # out0 = -s   (strided write into interleaved out tile)
                nc.vector.tensor_scalar_mul(ot4[:, :, :, 0], s, -1.0)
                # out1 = d - s  (strided write)
                nc.vector.tensor_sub(ot4[:, :, :, 1], dv, s)

                nc.sync.dma_start(
                    out=out[bt * P : (bt + 1) * P, :], in_=ot
                )

    nc.finalize()
    return nc


_NC_CACHE = None


def _get_nc():
    global _NC_CACHE
    if _NC_CACHE is None:
        _NC_CACHE = build_nc()
    return _NC_CACHE


def kernel(x, W, b):
    x = np.ascontiguousarray(np.asarray(x, dtype=np.float32))
    W = np.ascontiguousarray(np.asarray(W, dtype=np.float32))
    b = np.ascontiguousarray(np.asarray(b, dtype=np.float32))
    assert x.shape == (B, D) and W.shape == (C, 2, D) and b.shape == (C, 2)

    nc = _get_nc()
    w2d = W.reshape(2 * C, D)
    b2d = b.reshape(1, 2 * C)
    in_maps = [
        {"x": x[i * BS : (i + 1) * BS], "w": w2d, "b": b2d} for i in range(NCORES)
    ]
    res = run_bass_kernel_spmd(nc, in_maps, core_ids=list(range(NCORES)))
    full = np.concatenate([res.results[i]["out"] for i in range(NCORES)], axis=0)
    return full.reshape(B, C, 2)


# revision 39
# speedup vs baseline: 1.1203x; 1.0000x over previous
"""Trainium2 Bass kernel for nn_Classifier (per-class binary log_softmax head).

Reference computation:
    logits[b, c, t] = x[b, :] @ W[c, t, :] + bias[c, t]      # [B, C, 2]
    out = log_softmax(logits, axis=-1)

Key algebraic reduction: log_softmax over the 2 logits per class depends only
on the difference d = l1 - l0:
    out0 = -softplus(d)
    out1 = d - softplus(d)
where d[b, c] = x[b, :] @ (W[c,1,:] - W[c,0,:]) + (bias[c,1] - bias[c,0]).
This halves the matmul FLOPs vs computing both logits.

Strategy (8 NeuronCores, data-parallel over batch):
  - core i gets x rows [i*2048, (i+1)*2048); W and b are replicated.
  - on-device: dW = W1 - W0 (DVE), PE-transpose to [D, C] bf16;
    db = b1 - b0 folded into PSUM accumulation via a K=1 ones-matmul.
  - per 128-row batch tile: PE-transpose x chunks to [D, 128] bf16 (lhsT),
    matmul accumulate d in PSUM fp32, then ACT softplus + DVE/ACT epilogue
    writes the interleaved [128, 2000] fp32 output tile, DMA'd out (1 MiB).
"""

import os
import sys

for _p in ("/opt/trn_rl_repo", "/root/.axon_site/_ro/trn_rl_repo"):
    if os.path.isdir(_p) and _p not in sys.path:
        sys.path.insert(0, _p)

import numpy as np

import concourse.bass as bass
import concourse.mybir as mybir
import concourse.tile as tile
from concourse import bacc
from concourse.bass_utils import run_bass_kernel_spmd
from concourse.masks import make_identity

def _patch_act_tables():
    """Force Exp and Ln activations into ONE ACT table set.

    The stock table-set assignment puts Exp and Ln in different sets, so
    alternating Exp/Ln reloads the 1.3us ACT function table before every
    activation (~82us serialized on the scalar engine for this kernel).
    natural_log_exp_and_others contains both (at the higher-accuracy
    400-point tables). Removing exp/ln from every OTHER set makes bacc's
    insert_act_table_loads fixpoint assign both to that one set; set ids
    stay aligned with the stock act_info.json, so walrus adopts the
    pre-placed loads unchanged.
    """
    import functools

    import concourse.bacc as _bacc
    import concourse.hw_specs as _hw

    orig = _hw.get_activation_tables

    @functools.cache
    def patched(module_arch):
        exp = mybir.ActivationFunctionType.Exp
        ln = mybir.ActivationFunctionType.Ln
        out = {}
        for name, funcs in orig(module_arch).items():
            if name != "natural_log_exp_and_others":
                funcs = funcs - {exp, ln}
            out[name] = funcs
        return out

    _hw.get_activation_tables = patched
    _bacc.get_activation_tables = patched


_patch_act_tables()


# bump when the compile environment changes semantics: the neuron compile
# cache keys on the BIR bytes, and this tag is embedded in a tensor name so
# the key changes with it.
KERNEL_TAG = "v12"

P = 128
D = 512  # input dim
C = 1000  # num classes
B = 16384  # batch
NCORES = 8
BS = B // NCORES  # 2048 rows per core
BT = BS // P  # 16 batch tiles per core
KC = D // P  # 4 contraction chunks
NCH = 500  # classes per matmul n-chunk (2 chunks; 500 fp32 <= 1 PSUM bank)

F32 = mybir.dt.float32
BF16 = mybir.dt.bfloat16

# matmul operand dtype: bf16 is full PE rate (fp32 costs 2 passes). PSUM
# accumulation is fp32 either way. Set to F32 if accuracy requires it.
MM_DT = BF16


def build_nc():
    nc = bacc.Bacc(None, target_bir_lowering=False)
    x_in = nc.dram_tensor("x", [BS, D], F32, kind="ExternalInput").ap()
    w_in = nc.dram_tensor("w", [2 * C, D], F32, kind="ExternalInput").ap()
    b_in = nc.dram_tensor("b", [1, 2 * C], F32, kind="ExternalInput").ap()
    out = nc.dram_tensor("out", [BS, 2 * C], F32, kind="ExternalOutput").ap()

    with tile.TileContext(nc) as tc:
        with (
            tc.tile_pool(name="const", bufs=1) as const,
            tc.tile_pool(name="wstage", bufs=4) as wstage,
            tc.tile_pool(name="dwstage", bufs=2) as dwstage,
            tc.tile_pool(name="xstage", bufs=4) as xstage,
            tc.tile_pool(name="xtp", bufs=3) as xtp,
            tc.tile_pool(name="spool", bufs=4) as spool,
            tc.tile_pool(name="opool", bufs=3) as opool,
            tc.tile_pool(name="tpsum", bufs=2, space="PSUM") as tpsum,
            tc.tile_pool(name="mpsum", bufs=3, space="PSUM") as mpsum,
        ):
            identity = const.tile([P, P], MM_DT, name=f"identity_{KERNEL_TAG}")
            make_identity(nc, identity)

            ones_row = const.tile([1, P], MM_DT)
            nc.vector.memset(ones_row, 1.0)

            # ---- bias prep: db[c] = b[c,1] - b[c,0] ----
            btile = const.tile([1, 2 * C], F32)
            nc.sync.dma_start(out=btile, in_=b_in)
            b3 = btile.rearrange("p (c t) -> p t c", t=2)  # [1, 2, C] view
            db_f = const.tile([1, C], F32)
            nc.gpsimd.tensor_sub(db_f, b3[:, 1, :], b3[:, 0, :])
            db = const.tile([1, C], MM_DT)
            nc.gpsimd.tensor_copy(out=db, in_=db_f)

            # ---- dW prep: dwt[j][d_chunk][:, c] = (W1 - W0).T in MM_DT ----
            # split by n-chunk so chunk-0 matmuls start after half the W load
            dwt0 = const.tile([P, KC, NCH], MM_DT)
            dwt1 = const.tile([P, KC, NCH], MM_DT)
            dwts = [dwt0, dwt1]
            w3 = w_in.rearrange("(c t) d -> t c d", t=2)  # [2, C, D] view
            NWT = (C + P - 1) // P  # 8 row tiles (last = 104 rows)

            def prep_wtile(wt):
                r0 = wt * P
                rows = min(P, C - r0)
                w1t = wstage.tile([P, D], F32, tag="wst", name="w1t")
                w0t = wstage.tile([P, D], F32, tag="wst", name="w0t")
                nc.sync.dma_start(out=w1t[:rows], in_=w3[1, r0 : r0 + rows, :])
                nc.sync.dma_start(out=w0t[:rows], in_=w3[0, r0 : r0 + rows, :])
                # fused sub + bf16 cast (output dtype converts on write)
                dwb = dwstage.tile([P, D], MM_DT, tag="dwb", name="dwb")
                nc.gpsimd.tensor_sub(dwb[:rows], w1t[:rows], w0t[:rows])
                # 4 transposed chunks share one PSUM bank -> single wide cast
                pt = tpsum.tile([P, KC * P], MM_DT, tag="tp", name="ptw")
                for k in range(KC):
                    nc.tensor.transpose(
                        pt[:, k * P : k * P + rows],
                        dwb[:rows, k * P : (k + 1) * P],
                        identity[:rows, :rows],
                    )
                ptv = pt.rearrange("p (k b) -> p k b", k=KC)
                # scatter the [r0, r0+rows) class range into dwt0/dwt1
                for j in (0, 1):
                    lo = max(r0, j * NCH)
                    hi = min(r0 + rows, (j + 1) * NCH)
                    if lo < hi:
                        nc.vector.tensor_copy(
                            out=dwts[j][:, :, lo - j * NCH : hi - j * NCH],
                            in_=ptv[:, :, lo - r0 : hi - r0],
                        )

            # W tiles 0-3 cover the classes chunk-0 matmuls need; tiles 4-7
            # are emitted inside batch-tile 0 (after its chunk-0 matmuls) so
            # the in-order PE queue reaches the first matmul ~6us earlier.
            for wt in range(4):
                prep_wtile(wt)

            # ---- main loop over batch tiles ----
            for bt in range(BT):
                # scalar-engine HWDGE ring: x loads don't queue behind the
                # 4 MiB W load on the sync ring
                xt_ = xstage.tile([P, D], F32)
                nc.scalar.dma_start(out=xt_, in_=x_in[bt * P : (bt + 1) * P, :])
                xb = xstage.tile([P, D], MM_DT, tag="xb")
                nc.vector.tensor_copy(out=xb, in_=xt_)
                # 4 transposed chunks share one PSUM bank -> single wide cast
                ptx = tpsum.tile([P, KC * P], MM_DT, tag="tp")
                for k in range(KC):
                    nc.tensor.transpose(
                        ptx[:, k * P : (k + 1) * P],
                        xb[:, k * P : (k + 1) * P],
                        identity,
                    )
                xT = xtp.tile([P, KC, P], MM_DT, tag="xT")
                nc.vector.tensor_copy(
                    out=xT.rearrange("p k b -> p (k b)"), in_=ptx
                )

                ot = opool.tile([P, 2 * C], F32)
                # [P, j, c, t] view: class index = j*NCH + c, logit t
                ot4 = ot.rearrange("p (j c t) -> p j c t", j=2, t=2)

                # bank-aligned [P, 2, 512] PSUM tile; each matmul group fills
                # its own 512-wide bank (500 used), the epilogue then runs
                # both halves in single wide ops to amortize fixed overheads.
                dps = mpsum.tile([P, 2, 512], F32, tag="mp")
                for j in range(C // NCH):  # 2 n-chunks of 500 classes
                    c0 = j * NCH
                    dpsj = dps[:, j, :NCH]
                    for k in range(KC):
                        nc.tensor.matmul(
                            dpsj,
                            lhsT=xT[:, k, :],
                            rhs=dwts[j][:, k, :],
                            start=(k == 0),
                            stop=False,
                        )
                    # += ones.T @ db  (broadcasts bias diff across partitions)
                    nc.tensor.matmul(
                        dpsj,
                        lhsT=ones_row,
                        rhs=db[:, c0 : c0 + NCH],
                        start=False,
                        stop=True,
                    )
                    if bt == 0 and j == 0:
                        for wt in range(4, NWT):
                            prep_wtile(wt)
                # softplus(d) = ln(exp(d) + 1); this toolchain has no direct
                # Softplus ACT table, but natural_log_exp_and_others provides
                # Exp and Ln in one set. |d| <~ 6 so exp is safe.
                dv = dps[:, :, :NCH]  # [P, 2, 500]
                e = spool.tile([P, 2, NCH], F32, tag="sp")
                nc.scalar.activation(e, dv, mybir.ActivationFunctionType.Exp)
                s = spool.tile([P, 2, NCH], F32, tag="sp")
                nc.scalar.activation(
                    s, e, mybir.ActivationFunctionType.Ln, bias=1.0
                )
                # out0 = -s   (strided write into interleaved out tile)
                nc.vector.tensor_scalar_mul(ot4[:, :, :, 0], s, -1.0)
                # out1 = d - s  (strided write)
                nc.vector.tensor_sub(ot4[:, :, :, 1], dv, s)

                nc.sync.dma_start(
                    out=out[bt * P : (bt + 1) * P, :], in_=ot
                )

    nc.finalize()
    return nc


_NC_CACHE = None


def _get_nc():
    global _NC_CACHE
    if _NC_CACHE is None:
        _NC_CACHE = build_nc()
    return _NC_CACHE


def kernel(x, W, b):
    x = np.ascontiguousarray(np.asarray(x, dtype=np.float32))
    W = np.ascontiguousarray(np.asarray(W, dtype=np.float32))
    b = np.ascontiguousarray(np.asarray(b, dtype=np.float32))
    assert x.shape == (B, D) and W.shape == (C, 2, D) and b.shape == (C, 2)

    nc = _get_nc()
    w2d = W.reshape(2 * C, D)
    b2d = b.reshape(1, 2 * C)
    in_maps = [
        {"x": x[i * BS : (i + 1) * BS], "w": w2d, "b": b2d} for i in range(NCORES)
    ]
    res = run_bass_kernel_spmd(nc, in_maps, core_ids=list(range(NCORES)))
    full = np.concatenate([res.results[i]["out"] for i in range(NCORES)], axis=0)
    return full.reshape(B, C, 2)
